# revision 5
# baseline (speedup 1.0000x reference)
"""GNN message-passing layer (ConvolutionLayer) on 8 Trainium2 NeuronCores.

Reference computation (per graph b):
    deg[i]   = sum_j adj[b,i,j]
    agg      = (adj / deg) @ node_mat            # [N, Fin]
    out      = leaky_relu(agg @ W.T + b, 0.01)   # [N, Fout]

The graded metric is the wall time of kernel(**inputs) — which is dominated
by the axon tunnel (~50 MB/s host<->device) and one-time compiles, not by
device execution (~80 us/core).  So the design minimizes bytes moved and
per-call host work:

  * adj crosses the tunnel as uint8 (round(adj*255)): the constant scale
    cancels exactly in (adj/deg) @ x, integer values 0..255 are exact in
    bf16, and the quantization noise averages out over the 1024-term
    row-sums (measured ~4e-3 scale-rel absmax, gate is 2e-2).  On-chip the
    DVE up-converts u8 -> bf16 ahead of the TensorEngine.
  * node_mat ships as bf16 with a ones-column appended (col F of MM1's
    output is then the row degree), plus TWO packed extra rows: row 512 is
    W^T (bf16) and row 513 broadcasts b — so there are only two input
    tensors and 16 async per-core device_puts that pipeline in the tunnel.
  * the output is stored transposed as bf16 with the same [128, 514, 129]
    shape/dtype as x_in, so the already-resident x array doubles as the
    (unused) output-slot operand of bass_exec — no zero buffers ever cross
    the tunnel.  PJRT allocates the real result buffer fresh; every element
    the host reads is written by the kernel.
  * dispatch is an AOT-compiled shard_map executable fed NamedSharding-
    committed shards (no resharding "multi_slice" programs), and the Bass
    build + walrus NEFF compile run on the main thread while worker threads
    quantize/block the inputs and stream them to the devices.

Device kernel (per core, 8 graphs), mirroring the proven bf16 pipeline:
  MM1  P[i,c] = At_tile.T @ X'_tile accumulated over jt (PSUM fp32);
  agg  = P[:, :F] * reciprocal(P[:, F]) on DVE (bf16 out);
  PE transpose -> MM2 out^T = W @ agg^T (bf16, fp32 PSUM);
  ACT Lrelu(po + b) with per-partition bias writes bf16 straight into the
  129-wide output tile (col 128 is dead), stored via the idle GpSimd SWDGE.

Host-side DRAM layouts (partition-blocked so DMAs move multi-KB runs):
  at_in [128, BPC, NT, N] u8 : at_in[p,g,jt,i] = round(adj[g, i, jt*128+p]*255)
  x_in  [128, 514, F+1] bf16 : rows 0..511 node features (+ones col),
                               row 512 = W^T, row 513 = b (broadcast)
  o_out [128, 514, F+1] bf16 : o_out[o, g*NT+it, i] = out[g, it*128+i, o]
"""

import numpy as np
import ml_dtypes
from concurrent.futures import ThreadPoolExecutor

import concourse.mybir as mybir
import concourse.tile as tile
from concourse import bacc
from concourse.masks import make_identity

N_CORES = 8
B, N, F = 64, 1024, 128
BPC = B // N_CORES          # graphs per core
NT = N // 128               # 128-row tiles per graph
XROWS = BPC * NT + 2        # node-feature rows + packed W^T row + b row
LEAKY_SLOPE = 0.01

U8 = mybir.dt.uint8
BF16 = mybir.dt.bfloat16
F32 = mybir.dt.float32
BF16_NP = ml_dtypes.bfloat16

_CACHE = {}


def build_nc(repeat=None):
    """Build + compile the per-core kernel. `repeat` (benchmark only) wraps
    the whole body in a hardware For_i loop so device time can be measured
    as a slope over repeat counts, amortizing dispatch/tunnel overhead."""
    nc = bacc.Bacc(
        "TRN2", target_bir_lowering=False, debug=False, num_devices=N_CORES
    )
    at_d = nc.dram_tensor(
        "at_in", [128, BPC, NT, N], U8, kind="ExternalInput"
    ).ap()
    x_d = nc.dram_tensor(
        "x_in", [128, XROWS, F + 1], BF16, kind="ExternalInput"
    ).ap()
    o_d = nc.dram_tensor(
        "o_out", [128, XROWS, F + 1], BF16, kind="ExternalOutput"
    ).ap()

    with tile.TileContext(nc) as tc:
        with (
            tc.tile_pool(name="consts", bufs=1) as consts,
            tc.tile_pool(name="xp", bufs=2) as xp,
            tc.tile_pool(name="a8q", bufs=4) as a8q,
            tc.tile_pool(name="a8w", bufs=3) as a8w,
            tc.tile_pool(name="abq", bufs=4) as abq,
            tc.tile_pool(name="abw", bufs=3) as abw,
            tc.tile_pool(name="work", bufs=8) as work,
            tc.tile_pool(name="obig", bufs=4) as obig,
            tc.tile_pool(name="psp", bufs=4, space="PSUM") as psp,
            tc.tile_pool(name="pst", bufs=2, space="PSUM") as pst,
            tc.tile_pool(name="pso", bufs=2, space="PSUM") as pso,
        ):
            # consts ride the ACT DGE queue so the sync queue's first entries
            # are graph 0's x/At chunks (PE start gates on those).
            wt_mm2 = consts.tile([F, 1, F], BF16)  # W^T, MM2 stationary
            nc.scalar.dma_start(wt_mm2[:], x_d[:, BPC * NT : BPC * NT + 1, 0:F])
            bb_bf = consts.tile([F, 1, 1], BF16)
            nc.scalar.dma_start(bb_bf[:], x_d[:, BPC * NT + 1 : BPC * NT + 2, 0:1])
            bb32 = consts.tile([F, 1], F32)
            nc.vector.tensor_copy(bb32[:], bb_bf[:, 0, :])
            ident = consts.tile([128, 128], BF16)
            make_identity(nc, ident[:])

            def body(_it=None):
                for g in range(BPC):
                    x_g = xp.tile(
                        [128, NT, F + 1], BF16, name=f"x_{g}", tag="x"
                    )
                    nc.sync.dma_start(
                        x_g[:], x_d[:, g * NT : (g + 1) * NT, :]
                    )
                    # graph 0's At arrives in quarters so the first matmuls
                    # start a few us after launch; later graphs load whole
                    # (one descriptor per partition).  All input DMAs stay on
                    # the SP HWDGE queue (ACT-issued input DMAs can deadlock
                    # against pool-slot releases that need ACT work).
                    n_chunks = 4 if g == 0 else (2 if g == 1 else 1)
                    csz = NT // n_chunks
                    pool8 = a8q if g <= 1 else a8w
                    poolb = abq if g <= 1 else abw
                    at_chunks = []
                    for h in range(n_chunks):
                        at8 = pool8.tile(
                            [128, csz, N], U8, name=f"a8_{g}_{h}",
                            tag=f"a8{csz}",
                        )
                        nc.sync.dma_start(
                            at8[:], at_d[:, g, h * csz : (h + 1) * csz]
                        )
                        atb = poolb.tile(
                            [128, csz, N], BF16, name=f"ab_{g}_{h}",
                            tag=f"ab{csz}",
                        )
                        nc.vector.tensor_copy(atb[:], at8[:])
                        at_chunks.append(atb)

                    # one whole-graph output tile (1 DMA, 128 descriptors);
                    # the last graph stores in quarters to shorten the tail.
                    n_osplit = 4 if g == BPC - 1 else 1
                    osz = NT // n_osplit
                    o_parts = [
                        obig.tile(
                            [128, osz, F + 1], BF16, name=f"ob_{g}_{h}",
                            tag=f"ob{osz}",
                        )
                        for h in range(n_osplit)
                    ]

                    for i in range(NT):
                        o_big, io = o_parts[i // osz], i % osz
                        p = psp.tile([128, F + 1], F32, name=f"p_{g}_{i}", tag="p")
                        for jt in range(NT):
                            nc.tensor.matmul(
                                p[:],
                                at_chunks[jt // csz][
                                    :, jt % csz, i * 128 : (i + 1) * 128
                                ],
                                x_g[:, jt, :],
                                start=(jt == 0),
                                stop=(jt == NT - 1),
                            )
                        invd = work.tile(
                            [128, 1], F32, name=f"invd_{g}_{i}", tag="invd"
                        )
                        nc.vector.reciprocal(invd[:], p[:, F : F + 1])
                        agg = work.tile(
                            [128, F], BF16, name=f"agg_{g}_{i}", tag="agg"
                        )
                        nc.vector.tensor_scalar_mul(agg[:], p[:, 0:F], invd[:])

                        pt = pst.tile([128, 128], BF16, name=f"pt_{g}_{i}", tag="pt")
                        nc.tensor.transpose(pt[:], agg[:], ident[:])
                        aggt = work.tile(
                            [128, 128], BF16, name=f"aggt_{g}_{i}", tag="aggt"
                        )
                        nc.scalar.copy(aggt[:], pt[:])

                        # out^T[o, i] = W @ agg^T: W^T is the stationary, so
                        # the bias lands on the partition dim and fuses into
                        # one ACT op: leaky_relu(po + b) via Lrelu, writing
                        # bf16 into the 129-wide output tile (col 128 dead).
                        po = pso.tile([128, F], F32, name=f"po_{g}_{i}", tag="po")
                        nc.tensor.matmul(
                            po[:], wt_mm2[:, 0, :], aggt[:], start=True, stop=True
                        )
                        nc.scalar.activation(
                            o_big[:, io, 0:F],
                            po[:],
                            mybir.ActivationFunctionType.Lrelu,
                            bias=bb32[:],
                            alpha=LEAKY_SLOPE,
                        )
                        if io == osz - 1:
                            # output stores ride the idle GpSimd SWDGE queue so
                            # they never block input prefetch on the HWDGEs.
                            nc.gpsimd.dma_start(
                                o_d[
                                    :,
                                    g * NT + (i // osz) * osz
                                    : g * NT + (i // osz + 1) * osz,
                                ],
                                o_big[:],
                            )

            if repeat is None:
                body()
            else:
                with tc.For_i(0, repeat, 1) as it:
                    body(it)

    nc.compile()
    return nc


def get_nc():
    if "nc" not in _CACHE:
        _CACHE["nc"] = build_nc()
    return _CACHE["nc"]


def _get_pool():
    if "pool" not in _CACHE:
        _CACHE["pool"] = ThreadPoolExecutor(max_workers=N_CORES)
    return _CACHE["pool"]


def _block_at(adj_core):
    """[BPC, N(i), N(j)] f32 -> [128(p), BPC, NT, N(i)] u8 where
    out[p, g, jt, i] = round(adj[g, i, jt*128 + p] * 255)."""
    q = adj_core * np.float32(255.0)
    q += np.float32(0.5)
    q = q.astype(np.uint8)                         # truncate == round-half-up
    a = q.reshape(BPC, N, NT, 128)                 # [g, i, jt, p]
    out = np.empty((128, BPC, NT, N), np.uint8)
    out[...] = a.transpose(3, 0, 2, 1)             # [p, g, jt, i]
    return out


def _block_x(x_core, W, b):
    """[BPC, N(j), F] f32 -> [128(p), XROWS, F+1] bf16 with ones column,
    W^T packed in row BPC*NT and b broadcast in row BPC*NT+1."""
    xb = np.empty((128, XROWS, F + 1), BF16_NP)
    xv = xb[:, : BPC * NT, :].reshape(128, BPC, NT, F + 1)
    x = x_core.reshape(BPC, NT, 128, F)            # [g, jt, p, f]
    xv[:, :, :, :F] = x.transpose(2, 0, 1, 3)
    xv[:, :, :, F] = 1.0
    xb[:, BPC * NT, :F] = W.T
    xb[:, BPC * NT, F] = 0.0
    xb[:, BPC * NT + 1, :] = np.asarray(b, np.float32)[:, None]
    return xb


def make_in_maps(node_mat, adj_mat, W, b):
    """Per-core numpy input dicts (used by the slope-bench harness)."""
    node_mat, adj_mat = np.asarray(node_mat), np.asarray(adj_mat)
    W, b = np.asarray(W), np.asarray(b)
    return [
        {
            "at_in": _block_at(adj_mat[c * BPC : (c + 1) * BPC]),
            "x_in": _block_x(node_mat[c * BPC : (c + 1) * BPC], W, b),
        }
        for c in range(N_CORES)
    ]


def _get_exec():
    """AOT-compile the sharded bass_exec program (cached). Safe to run on
    the main thread while worker threads stream inputs to the devices."""
    if "exec" in _CACHE:
        return _CACHE["exec"]
    import jax
    from jax.sharding import Mesh, PartitionSpec, NamedSharding
    try:
        from jax.experimental.shard_map import shard_map
    except ImportError:
        from jax import shard_map
    from concourse.bass2jax import (
        _bass_exec_p,
        install_neuronx_cc_hook,
        partition_id_tensor,
    )

    install_neuronx_cc_hook()
    nc = get_nc()
    partition_name = (
        nc.partition_id_tensor.name if nc.partition_id_tensor else None
    )
    in_names, out_names, out_avals = [], [], []
    for alloc in nc.m.functions[0].allocations:
        if not isinstance(alloc, mybir.MemoryLocationSet):
            continue
        name = alloc.memorylocations[0].name
        if alloc.kind == "ExternalInput":
            if name != partition_name:
                in_names.append(name)
        elif alloc.kind == "ExternalOutput":
            out_names.append(name)
            out_avals.append(
                jax.core.ShapedArray(
                    tuple(alloc.tensor_shape), mybir.dt.np(alloc.dtype)
                )
            )
    all_in = in_names + out_names + ([partition_name] if partition_name else [])
    param_names = in_names + out_names  # jit params, in operand order

    def _body(*args):
        operands = list(args)
        if partition_name is not None:
            operands.append(partition_id_tensor())
        return tuple(
            _bass_exec_p.bind(
                *operands,
                out_avals=tuple(out_avals),
                in_names=tuple(all_in),
                out_names=tuple(out_names),
                lowering_input_output_aliases=(),
                sim_require_finite=True,
                sim_require_nnan=True,
                nc=nc,
            )
        )

    devices = jax.devices()[:N_CORES]
    mesh = Mesh(np.asarray(devices), ("core",))
    sh = NamedSharding(mesh, PartitionSpec("core"))
    spec = (PartitionSpec("core"),)
    fn = jax.jit(
        shard_map(
            _body,
            mesh=mesh,
            in_specs=spec * len(param_names),
            out_specs=spec * len(out_names),
            check_rep=False,
        )
    )
    shapes = {
        "at_in": ((N_CORES * 128, BPC, NT, N), np.uint8),
        "x_in": ((N_CORES * 128, XROWS, F + 1), BF16_NP),
        "o_out": ((N_CORES * 128, XROWS, F + 1), BF16_NP),
    }
    structs = [
        jax.ShapeDtypeStruct(*shapes[n], sharding=sh) for n in param_names
    ]
    compiled = fn.lower(*structs).compile()
    _CACHE["exec"] = (compiled, devices, sh, param_names)
    return _CACHE["exec"]


def kernel(node_mat, adj_mat, W, b):
    import jax

    node_mat = np.asarray(node_mat, np.float32)
    adj_mat = np.asarray(adj_mat, np.float32)
    W = np.asarray(W, np.float32)
    b = np.asarray(b, np.float32)

    devices = jax.devices()[:N_CORES]
    pool = _get_pool()

    def put_at(c):
        blk = _block_at(adj_mat[c * BPC : (c + 1) * BPC])
        return jax.device_put(blk, devices[c])

    def put_x(c):
        blk = _block_x(node_mat[c * BPC : (c + 1) * BPC], W, b)
        return jax.device_put(blk, devices[c])

    # First contact with a device must be a small SERIAL put: concurrent
    # large first-contact transfers collapse the tunnel (~60x slowdown
    # measured).  Warm each device, then submit its bulk jobs; the at
    # shards are ~4x the x bytes so they are queued first.
    at_futs, x_futs = [None] * N_CORES, [None] * N_CORES
    if not _CACHE.get("warmed"):
        tiny = np.zeros(1024, np.uint8)
        for c in range(N_CORES):
            jax.device_put(tiny, devices[c]).block_until_ready()
            at_futs[c] = pool.submit(put_at, c)
        _CACHE["warmed"] = True
    else:
        at_futs = [pool.submit(put_at, c) for c in range(N_CORES)]
    x_futs = [pool.submit(put_x, c) for c in range(N_CORES)]

    # Build + compile on the main thread, overlapped with the transfers.
    compiled, devices, sh, param_names = _get_exec()

    at_g = jax.make_array_from_single_device_arrays(
        (N_CORES * 128, BPC, NT, N), sh, [f.result() for f in at_futs]
    )
    x_g = jax.make_array_from_single_device_arrays(
        (N_CORES * 128, XROWS, F + 1), sh, [f.result() for f in x_futs]
    )
    arrays = {"at_in": at_g, "x_in": x_g, "o_out": x_g}
    (out,) = compiled(*[arrays[n] for n in param_names])

    # Fetch per-shard and unblock concurrently: o_core[o, g*NT+it, i] =
    # out[g, it*128+i, o]; drop the dead col 128 and the 2 packed rows.
    res = np.empty((B, N, F), np.float32)

    def fetch_unblock(shard):
        c = shard.index[0].start // 128
        data = np.asarray(shard.data)              # [128, XROWS, 129] bf16
        core = data[:, : BPC * NT, :F].reshape(128, BPC, NT, F)
        dst = res[c * BPC : (c + 1) * BPC].reshape(BPC, NT, 128, F)
        dst[...] = core.transpose(1, 2, 3, 0)      # casts bf16 -> f32
        return c

    list(pool.map(fetch_unblock, out.addressable_shards))
    return res
